# revision 1
# baseline (speedup 1.0000x reference)
"""nn_MultiHeadAttention: fused MHA + residual + LayerNorm on 8 TRN2 NeuronCores.

Sharding: core = (batch b, query-half). Each core computes, for its batch:
  - Q projection for its 512 query rows, K/V projections for all 1024 keys
    (K/V work duplicated within a batch pair -> zero cross-core communication),
  - all 16 heads' attention for its query rows,
  - output projection + residual + LayerNorm for its rows.
Host concatenates the 8 [512, 1024] results into [4, 1024, 1024].

Schedule (single TileContext dataflow program, issue order = priority):
  K-proj (4 passes) -> Q-proj (2 passes) -> per head-pair: scores (row-packed
  2 heads across the 64-row PE groups) -> exp (ACT) -> PV, with the four
  V-projection passes and the O-projection interleaved into the attention
  stream so the ~70us of ACT exp work hides behind PE matmuls.

DMA: one dma_start per [128, 512..1024] chunk (2-4KB per partition line),
split across both HWDGE queues: SP carries weights + output, ACT carries
activations.  All matmuls f32r (1 cycle/row at N=512).
"""
import numpy as np

import concourse.bass as bass
import concourse.mybir as mybir
import concourse.tile as tile
from concourse import bacc, bass_utils

B, S, D, H, DK = 4, 1024, 1024, 16, 64
P = 128
SH = S // 2           # query rows per core
NC = D // P           # 8 chunks of 128 along any d-dimension
NP = H // 2           # 8 head pairs (one 128-dim chunk each)
NCORES = 8
EPS = 1e-6
f32 = mybir.dt.float32
f32r = mybir.dt.float32r

TRACE = False          # set by test.py to profile
LAST_EXEC_NS = None

_CACHE = {}


def _build():
    nc = bacc.Bacc("TRN2")
    xqT = nc.dram_tensor("xqT", [D, SH], f32, kind="ExternalInput")
    xkT = nc.dram_tensor("xkT", [D, S], f32, kind="ExternalInput")
    xvT = nc.dram_tensor("xvT", [D, S], f32, kind="ExternalInput")
    wq = nc.dram_tensor("wq", [D, D], f32, kind="ExternalInput")   # Wq.T  [in, out]
    wk = nc.dram_tensor("wk", [D, D], f32, kind="ExternalInput")
    wv = nc.dram_tensor("wv", [D, D], f32, kind="ExternalInput")
    wo = nc.dram_tensor("wo", [D, D], f32, kind="ExternalInput")   # Wo.T  [d, e]
    resid = nc.dram_tensor("resid", [SH, D], f32, kind="ExternalInput")
    gamma = nc.dram_tensor("gamma", [D], f32, kind="ExternalInput")
    beta = nc.dram_tensor("beta", [D], f32, kind="ExternalInput")
    out = nc.dram_tensor("out", [SH, D], f32, kind="ExternalOutput")

    with tile.TileContext(nc) as tc:
        with (
            tc.tile_pool(name="wpool", bufs=8) as wpool,
            tc.tile_pool(name="xs", bufs=10) as xsp,
            tc.tile_pool(name="xvp", bufs=8) as xvp,
            tc.tile_pool(name="persist", bufs=1) as persist,
            tc.tile_pool(name="expp", bufs=4) as expp,
            tc.tile_pool(name="rp", bufs=2) as rp,
            tc.tile_pool(name="small", bufs=2) as small,
            tc.tile_pool(name="psum", bufs=4, space="PSUM") as psum,
        ):
            # ---------------- persistent tiles ----------------
            kT = persist.tile([P, NC, S], f32r)       # [dim-in-pair, pair, sk]
            qT = persist.tile([P, NC, SH], f32r)      # [dim-in-pair, pair, sq]
            vt = persist.tile([P, NC, H, DK + 1], f32r)  # [sk-in-chunk, sk-chunk, (h, d|1)]
            xT = persist.tile([P, NC, SH], f32r)      # normalized attn out
            gb = persist.tile([P, 2, D], f32)         # gamma/beta broadcast
            eps_t = persist.tile([P, 1], f32)

            nc.vector.memset(eps_t, EPS)
            nc.vector.memset(vt[:, :, :, DK:DK + 1].bitcast(f32), 1.0)  # ones col

            def load_w(w, i, nm):
                """One [128, 1024] weight chunk as a single 512KB DMA (SP q)."""
                wt = wpool.tile([P, D], f32r, tag="w", name=f"{nm}{i}")
                nc.sync.dma_start(wt, w[i * P:(i + 1) * P, :].bitcast(f32r))
                return wt

            def load_xh(x, i, col0, nm):
                """One [128, 512] half-chunk as a single 256KB DMA (ACT q)."""
                xc = xsp.tile([P, SH], f32r, tag="xs", name=nm)
                nc.scalar.dma_start(
                    xc, x[i * P:(i + 1) * P, col0:col0 + SH].bitcast(f32r)
                )
                return xc

            def big():
                return psum.tile([P, 2, SH], f32, tag="mm", name="big")

            # ---------------- K projection ----------------
            # kT[j, sk] = sum_i Wk.T[i, j] * xkT[i, sk]
            # 4 passes (sk-half x j-group); xk half-chunks stay live across
            # their half's two j-group passes.
            wk8 = []
            for sh in range(2):
                xk8 = []
                for jg in range(2):
                    ps_k = [big() for _ in range(2)]
                    for i in range(NC):
                        if sh == 0 and jg == 0:
                            wk8.append(load_w(wk, i, "wk"))
                        if jg == 0:
                            xk8.append(load_xh(xkT, i, sh * SH, "xk"))
                        for jj in range(4):
                            j = jg * 4 + jj
                            nc.tensor.matmul(
                                ps_k[jj // 2][:, jj % 2, :],
                                wk8[i][:, j * P:(j + 1) * P], xk8[i],
                                start=(i == 0), stop=(i == NC - 1),
                            )
                    for jj in range(4):
                        j = jg * 4 + jj
                        eng = nc.scalar.copy if jj % 2 == 0 else nc.vector.tensor_copy
                        eng(kT[:, j, sh * SH:(sh + 1) * SH], ps_k[jj // 2][:, jj % 2, :])

            # ---------------- Q projection ----------------
            wq8 = []
            xq8 = []
            for jg in range(2):
                ps_q = [big() for _ in range(2)]
                for i in range(NC):
                    if jg == 0:
                        wq8.append(load_w(wq, i, "wq"))
                        xq8.append(load_xh(xqT, i, 0, "xq"))
                    for jj in range(4):
                        j = jg * 4 + jj
                        nc.tensor.matmul(
                            ps_q[jj // 2][:, jj % 2, :],
                            wq8[i][:, j * P:(j + 1) * P], xq8[i],
                            start=(i == 0), stop=(i == NC - 1),
                        )
                for jj in range(4):
                    j = jg * 4 + jj
                    eng = nc.scalar.copy if jj % 2 == 0 else nc.vector.tensor_copy
                    eng(qT[:, j, :], ps_q[jj // 2][:, jj % 2, :])

            # ---------------- V projection (4 passes) + attention ----------
            # V pass (scg, dh): vt[sk in half scg, heads dh*8..dh*8+7].
            # xv halves are re-loaded per dh pass (cheaper than keeping 4MB
            # live); wv chunks stay resident across all four passes.
            wv8 = []

            def v_pass(scg, dh):
                ps_v = [big() for _ in range(2)]
                xvh = []
                for i in range(NC):
                    if scg == 0 and dh == 0:
                        wv8.append(load_w(wv, i, "wv"))
                    xc = xvp.tile([P, SH], f32r, tag="xv", name="xv")
                    nc.scalar.dma_start(
                        xc, xvT[i * P:(i + 1) * P, scg * SH:scg * SH + SH].bitcast(f32r)
                    )
                    xvh.append(xc)
                    for sl in range(4):
                        nc.tensor.matmul(
                            ps_v[sl // 2][:, sl % 2, :],
                            xvh[i][:, sl * P:(sl + 1) * P],
                            wv8[i][:, dh * SH:(dh + 1) * SH],
                            start=(i == 0), stop=(i == NC - 1),
                        )
                for sl in range(4):
                    sc = scg * 4 + sl
                    nc.vector.tensor_copy(
                        vt[:, sc, dh * 8:(dh + 1) * 8, :DK],
                        ps_v[sl // 2][:, sl % 2, :].rearrange("p (h d) -> p h d", d=DK),
                    )

            pv_state = {}

            def sc_exp_cp(p, cp):
                """Row-packed scores for heads (2p, 2p+1), sk chunks
                (2cp, 2cp+1), then exp. Returns (etA, etB)."""
                ps2 = [big() for _ in range(2)]
                for k in range(2):
                    c = 2 * cp + k
                    for a in range(2):
                        nc.tensor.matmul(
                            ps2[a][:, k, :],
                            kT[a * DK:(a + 1) * DK, p, c * P:(c + 1) * P],
                            qT[a * DK:(a + 1) * DK, p, :],
                            start=True, stop=True,
                        )
                ets = []
                for a in range(2):
                    et = expp.tile([P, 2, SH], f32r, tag="e", name="et")
                    nc.scalar.activation(
                        out=et, in_=ps2[a],
                        func=mybir.ActivationFunctionType.Exp,
                        scale=1.0 / np.sqrt(np.float32(DK)),
                    )
                    ets.append(et)
                return ets

            def pv_cp(h, cp, et):
                """Accumulate PV for head h over sk chunks (2cp, 2cp+1) in
                bank h%2 of its pair's accumulator tile; normalize after the
                last chunk."""
                a, p = h % 2, h // 2
                if p not in pv_state:
                    pv_state[p] = big()
                pvt = pv_state[p]
                for k in range(2):
                    c = 2 * cp + k
                    nc.tensor.matmul(
                        pvt[:DK + 1, a, :], vt[:, c, h, :], et[:, k, :],
                        start=(c == 0), stop=(c == NC - 1),
                    )
                if cp != 3:
                    return
                if a == 1:
                    del pv_state[p]
                sums_raw = small.tile([1, SH], f32, tag="sums_raw", name="sums_raw",
                                      bufs=1)
                nc.vector.tensor_copy(sums_raw, pvt[DK:DK + 1, a, :])
                sums = small.tile([1, SH], f32, tag="sums", name="sums")
                # approx reciprocal needs an SBUF input (bit-trick path)
                nc.vector.reciprocal_approx_fast(sums, sums_raw)
                rbc = small.tile([DK, SH], f32, tag="rbc", name="rbc")
                nc.gpsimd.partition_broadcast(rbc, sums)
                nc.vector.tensor_mul(
                    out=xT[a * DK:(a + 1) * DK, p, :], in0=pvt[:DK, a, :], in1=rbc
                )

            def pair(p, cps):
                # software-pipelined: scores/exp for chunk-pair cp+1 are
                # emitted before PV of cp, so the PE's static order never
                # makes the next exp wait a full pv+scores round-trip
                prev = None
                for cp in cps:
                    ets = sc_exp_cp(p, cp)
                    if prev is not None:
                        pcp, (eA, eB) = prev
                        pv_cp(2 * p, pcp, eA)
                        pv_cp(2 * p + 1, pcp, eB)
                    prev = (cp, ets)
                pcp, (eA, eB) = prev
                pv_cp(2 * p, pcp, eA)
                pv_cp(2 * p + 1, pcp, eB)

            # pair 0 leads: its first two chunk-pairs' scores/exp issue before
            # v_pass(0,0) so ACT has work the moment vt lands; its PV drains
            # those quarters as soon as each vt half-pass completes.
            e00 = sc_exp_cp(0, 0)
            e01 = sc_exp_cp(0, 1)
            v_pass(0, 0)
            for cp, (eA, eB) in ((0, e00), (1, e01)):
                pv_cp(0, cp, eA)
                pv_cp(1, cp, eB)
            e02 = sc_exp_cp(0, 2)
            e03 = sc_exp_cp(0, 3)
            v_pass(1, 0)
            for cp, (eA, eB) in ((2, e02), (3, e03)):
                pv_cp(0, cp, eA)
                pv_cp(1, cp, eB)
            pair(1, range(4))
            v_pass(0, 1)
            pair(2, range(4))
            pair(3, range(4))
            v_pass(1, 1)
            for p in range(4, NP):
                pair(p, range(4))

            # ---------------- output projection + residual + LN ----------
            wo8 = [load_w(wo, i, "wo") for i in range(NC)]
            for i, t in enumerate((gamma, beta)):
                nc.scalar.dma_start(
                    gb[:, i, :], bass.AP(tensor=t, offset=0, ap=[[0, P], [1, D]])
                )
            for scc in range(4):
                rc = rp.tile([P, D], f32, tag="r", name="rc")
                nc.scalar.dma_start(rc, resid[scc * P:(scc + 1) * P, :])
                ps_o = big()
                for dc in range(NC):
                    for eh in range(2):
                        nc.tensor.matmul(
                            ps_o[:, eh, :],
                            xT[:, dc, scc * P:(scc + 1) * P],
                            wo8[dc][:, eh * SH:(eh + 1) * SH],
                            start=(dc == 0), stop=(dc == NC - 1),
                        )
                xl = rc  # LN runs in-place on the residual tile
                nc.vector.tensor_add(
                    out=xl, in0=ps_o.rearrange("p a b -> p (a b)"), in1=rc
                )
                stats = small.tile([P, 2, nc.vector.BN_STATS_DIM], f32, tag="stats",
                                   name="stats")
                for i in range(2):
                    nc.vector.bn_stats(stats[:, i, :], xl[:, i * SH:(i + 1) * SH])
                mv = small.tile([P, nc.vector.BN_AGGR_DIM], f32, tag="mv", name="mv")
                nc.vector.bn_aggr(mv, stats)
                std = small.tile([P, 1], f32, tag="std", name="std")
                nc.scalar.activation(
                    out=std, in_=mv[:, 1:2],
                    func=mybir.ActivationFunctionType.Sqrt,
                    bias=eps_t, scale=1.0,
                )
                rstd = small.tile([P, 1], f32, tag="rstd", name="rstd")
                nc.vector.reciprocal_approx_fast(rstd, std)
                nc.vector.tensor_scalar(
                    out=xl, in0=xl, scalar1=mv[:, 0:1], scalar2=rstd,
                    op0=mybir.AluOpType.subtract, op1=mybir.AluOpType.mult,
                )
                nc.vector.tensor_mul(out=xl, in0=xl, in1=gb[:, 0, :])
                nc.vector.tensor_add(out=xl, in0=xl, in1=gb[:, 1, :])
                nc.sync.dma_start(out[scc * P:(scc + 1) * P, :], xl)

    nc.compile()
    return nc


def kernel(query, key, value, Wq, Wk, Wv, Wo, ln_gamma, ln_beta):
    global LAST_EXEC_NS
    if "nc" not in _CACHE:
        _CACHE["nc"] = _build()
    nc = _CACHE["nc"]

    query = np.asarray(query, np.float32)
    key = np.asarray(key, np.float32)
    value = np.asarray(value, np.float32)
    wqT = np.ascontiguousarray(np.asarray(Wq, np.float32).T)
    wkT = np.ascontiguousarray(np.asarray(Wk, np.float32).T)
    wvT = np.ascontiguousarray(np.asarray(Wv, np.float32).T)
    woT = np.ascontiguousarray(np.asarray(Wo, np.float32).T)
    gamma = np.ascontiguousarray(np.asarray(ln_gamma, np.float32))
    beta = np.ascontiguousarray(np.asarray(ln_beta, np.float32))

    in_maps = []
    for core in range(NCORES):
        b, half = core // 2, core % 2
        sl = slice(half * SH, (half + 1) * SH)
        in_maps.append({
            "xqT": np.ascontiguousarray(query[b].T[:, sl]),
            "xkT": np.ascontiguousarray(key[b].T),
            "xvT": np.ascontiguousarray(value[b].T),
            "wq": wqT, "wk": wkT, "wv": wvT, "wo": woT,
            "resid": np.ascontiguousarray(query[b, sl]),
            "gamma": gamma, "beta": beta,
        })

    res = bass_utils.run_bass_kernel_spmd(
        nc, in_maps, core_ids=list(range(NCORES)), trace=TRACE
    )
    LAST_EXEC_NS = res.exec_time_ns

    out = np.empty((B, S, D), np.float32)
    for core in range(NCORES):
        b, half = core // 2, core % 2
        out[b, half * SH:(half + 1) * SH] = np.asarray(res.results[core]["out"])
    return out



# revision 2
# speedup vs baseline: 8.6509x; 8.6509x over previous
"""nn_MultiHeadAttention: fused MHA + residual + LayerNorm on 8 TRN2 NeuronCores.

Sharding: core = (batch b, query-half). Each core computes, for its batch:
  - Q projection for its 512 query rows, K/V projections for all 1024 keys
    (K/V work duplicated within a batch pair -> zero cross-core communication),
  - all 16 heads' attention for its query rows,
  - output projection + residual + LayerNorm for its rows.
Host concatenates the 8 [512, 1024] results into [4, 1024, 1024].

Schedule (single TileContext dataflow program, issue order = priority):
  K-proj (4 passes) -> Q-proj (2 passes) -> per head-pair: scores (row-packed
  2 heads across the 64-row PE groups) -> exp (ACT) -> PV, with the four
  V-projection passes and the O-projection interleaved into the attention
  stream so the ~70us of ACT exp work hides behind PE matmuls.

DMA: one dma_start per [128, 512..1024] chunk (2-4KB per partition line),
split across both HWDGE queues: SP carries weights + output, ACT carries
activations.  All matmuls f32r (1 cycle/row at N=512).
"""
import numpy as np

import concourse.bass as bass
import concourse.mybir as mybir
import concourse.tile as tile
from concourse import bacc, bass_utils

B, S, D, H, DK = 4, 1024, 1024, 16, 64
P = 128
SH = S // 2           # query rows per core
NC = D // P           # 8 chunks of 128 along any d-dimension
NP = H // 2           # 8 head pairs (one 128-dim chunk each)
NCORES = 8
EPS = 1e-6
f32 = mybir.dt.float32
f32r = mybir.dt.float32r

TRACE = False          # set by test.py to profile
LAST_EXEC_NS = None

_CACHE = {}


def _build(apply_gb):
    nc = bacc.Bacc("TRN2")
    xqT = nc.dram_tensor("xqT", [D, SH], f32, kind="ExternalInput")
    xkT = nc.dram_tensor("xkT", [D, S], f32, kind="ExternalInput")
    xvT = nc.dram_tensor("xvT", [D, S], f32, kind="ExternalInput")
    wq = nc.dram_tensor("wq", [D, D], f32, kind="ExternalInput")   # Wq.T  [in, out]
    wk = nc.dram_tensor("wk", [D, D], f32, kind="ExternalInput")
    wv = nc.dram_tensor("wv", [D, D], f32, kind="ExternalInput")
    wo = nc.dram_tensor("wo", [D, D], f32, kind="ExternalInput")   # Wo.T  [d, e]
    resid = nc.dram_tensor("resid", [SH, D], f32, kind="ExternalInput")
    gamma = nc.dram_tensor("gamma", [D], f32, kind="ExternalInput")
    beta = nc.dram_tensor("beta", [D], f32, kind="ExternalInput")
    out = nc.dram_tensor("out", [SH, D], f32, kind="ExternalOutput")

    with tile.TileContext(nc) as tc:
        with (
            tc.tile_pool(name="wpool", bufs=8) as wpool,
            tc.tile_pool(name="xs", bufs=10) as xsp,
            tc.tile_pool(name="xvp", bufs=8) as xvp,
            tc.tile_pool(name="persist", bufs=1) as persist,
            tc.tile_pool(name="expp", bufs=4) as expp,
            tc.tile_pool(name="rp", bufs=2) as rp,
            tc.tile_pool(name="small", bufs=2) as small,
            tc.tile_pool(name="psum", bufs=4, space="PSUM") as psum,
        ):
            # ---------------- persistent tiles ----------------
            kT = persist.tile([P, NC, S], f32r)       # [dim-in-pair, pair, sk]
            qT = persist.tile([P, NC, SH], f32r)      # [dim-in-pair, pair, sq]
            vt = persist.tile([P, NC, H, DK + 1], f32r)  # [sk-in-chunk, sk-chunk, (h, d|1)]
            xT = persist.tile([P, NC, SH], f32r)      # normalized attn out
            if apply_gb:
                gb = persist.tile([P, 2, D], f32)     # gamma/beta broadcast
            eps_t = persist.tile([P, 1], f32)

            nc.vector.memset(eps_t, EPS)
            nc.vector.memset(vt[:, :, :, DK:DK + 1].bitcast(f32), 1.0)  # ones col

            def load_w(w, i, nm):
                """One [128, 1024] weight chunk as a single 512KB DMA (SP q)."""
                wt = wpool.tile([P, D], f32r, tag="w", name=f"{nm}{i}")
                nc.sync.dma_start(wt, w[i * P:(i + 1) * P, :].bitcast(f32r))
                return wt

            def load_xh(x, i, col0, nm):
                """One [128, 512] half-chunk as a single 256KB DMA (ACT q)."""
                xc = xsp.tile([P, SH], f32r, tag="xs", name=nm)
                nc.scalar.dma_start(
                    xc, x[i * P:(i + 1) * P, col0:col0 + SH].bitcast(f32r)
                )
                return xc

            def big():
                return psum.tile([P, 2, SH], f32, tag="mm", name="big")

            # ---------------- K projection ----------------
            # kT[j, sk] = sum_i Wk.T[i, j] * xkT[i, sk]
            # 4 passes (sk-half x j-group); xk half-chunks stay live across
            # their half's two j-group passes.
            wk8 = []
            for sh in range(2):
                xk8 = []
                for jg in range(2):
                    ps_k = [big() for _ in range(2)]
                    for i in range(NC):
                        if sh == 0 and jg == 0:
                            wk8.append(load_w(wk, i, "wk"))
                        if jg == 0:
                            xk8.append(load_xh(xkT, i, sh * SH, "xk"))
                        for jj in range(4):
                            j = jg * 4 + jj
                            nc.tensor.matmul(
                                ps_k[jj // 2][:, jj % 2, :],
                                wk8[i][:, j * P:(j + 1) * P], xk8[i],
                                start=(i == 0), stop=(i == NC - 1),
                            )
                    for jj in range(4):
                        j = jg * 4 + jj
                        eng = nc.scalar.copy if jj % 2 == 0 else nc.vector.tensor_copy
                        eng(kT[:, j, sh * SH:(sh + 1) * SH], ps_k[jj // 2][:, jj % 2, :])

            # ---------------- Q projection ----------------
            wq8 = []
            xq8 = []
            for jg in range(2):
                ps_q = [big() for _ in range(2)]
                for i in range(NC):
                    if jg == 0:
                        wq8.append(load_w(wq, i, "wq"))
                        xq8.append(load_xh(xqT, i, 0, "xq"))
                    for jj in range(4):
                        j = jg * 4 + jj
                        nc.tensor.matmul(
                            ps_q[jj // 2][:, jj % 2, :],
                            wq8[i][:, j * P:(j + 1) * P], xq8[i],
                            start=(i == 0), stop=(i == NC - 1),
                        )
                for jj in range(4):
                    j = jg * 4 + jj
                    eng = nc.scalar.copy if jj % 2 == 0 else nc.vector.tensor_copy
                    eng(qT[:, j, :], ps_q[jj // 2][:, jj % 2, :])

            # ---------------- V projection (4 passes) + attention ----------
            # V pass (scg, dh): vt[sk in half scg, heads dh*8..dh*8+7].
            # xv halves are re-loaded per dh pass (cheaper than keeping 4MB
            # live); wv chunks stay resident across all four passes.
            wv8 = []

            def v_pass(scg, dh):
                ps_v = [big() for _ in range(2)]
                xvh = []
                for i in range(NC):
                    if scg == 0 and dh == 0:
                        wv8.append(load_w(wv, i, "wv"))
                    xc = xvp.tile([P, SH], f32r, tag="xv", name="xv")
                    nc.scalar.dma_start(
                        xc, xvT[i * P:(i + 1) * P, scg * SH:scg * SH + SH].bitcast(f32r)
                    )
                    xvh.append(xc)
                    for sl in range(4):
                        nc.tensor.matmul(
                            ps_v[sl // 2][:, sl % 2, :],
                            xvh[i][:, sl * P:(sl + 1) * P],
                            wv8[i][:, dh * SH:(dh + 1) * SH],
                            start=(i == 0), stop=(i == NC - 1),
                        )
                for sl in range(4):
                    sc = scg * 4 + sl
                    nc.vector.tensor_copy(
                        vt[:, sc, dh * 8:(dh + 1) * 8, :DK],
                        ps_v[sl // 2][:, sl % 2, :].rearrange("p (h d) -> p h d", d=DK),
                    )

            pv_state = {}

            def sc_exp_cp(p, cp):
                """Row-packed scores for heads (2p, 2p+1), sk chunks
                (2cp, 2cp+1), then exp. Returns (etA, etB)."""
                ps2 = [big() for _ in range(2)]
                for k in range(2):
                    c = 2 * cp + k
                    for a in range(2):
                        nc.tensor.matmul(
                            ps2[a][:, k, :],
                            kT[a * DK:(a + 1) * DK, p, c * P:(c + 1) * P],
                            qT[a * DK:(a + 1) * DK, p, :],
                            start=True, stop=True,
                        )
                ets = []
                for a in range(2):
                    et = expp.tile([P, 2, SH], f32r, tag="e", name="et")
                    nc.scalar.activation(
                        out=et, in_=ps2[a],
                        func=mybir.ActivationFunctionType.Exp,
                        scale=1.0 / np.sqrt(np.float32(DK)),
                    )
                    ets.append(et)
                return ets

            def pv_cp(h, cp, et):
                """Accumulate PV for head h over sk chunks (2cp, 2cp+1) in
                bank h%2 of its pair's accumulator tile; normalize after the
                last chunk."""
                a, p = h % 2, h // 2
                if p not in pv_state:
                    pv_state[p] = big()
                pvt = pv_state[p]
                for k in range(2):
                    c = 2 * cp + k
                    nc.tensor.matmul(
                        pvt[:DK + 1, a, :], vt[:, c, h, :], et[:, k, :],
                        start=(c == 0), stop=(c == NC - 1),
                    )
                if cp != 3:
                    return
                if a == 1:
                    del pv_state[p]
                sums_raw = small.tile([1, SH], f32, tag="sums_raw", name="sums_raw",
                                      bufs=1)
                nc.vector.tensor_copy(sums_raw, pvt[DK:DK + 1, a, :])
                sums = small.tile([1, SH], f32, tag="sums", name="sums")
                # approx reciprocal needs an SBUF input (bit-trick path)
                nc.vector.reciprocal_approx_fast(sums, sums_raw)
                rbc = small.tile([DK, SH], f32, tag="rbc", name="rbc")
                nc.gpsimd.partition_broadcast(rbc, sums)
                nc.vector.tensor_mul(
                    out=xT[a * DK:(a + 1) * DK, p, :], in0=pvt[:DK, a, :], in1=rbc
                )

            def pair(p, cps):
                # software-pipelined: scores/exp for chunk-pair cp+1 are
                # emitted before PV of cp, so the PE's static order never
                # makes the next exp wait a full pv+scores round-trip
                prev = None
                for cp in cps:
                    ets = sc_exp_cp(p, cp)
                    if prev is not None:
                        pcp, (eA, eB) = prev
                        pv_cp(2 * p, pcp, eA)
                        pv_cp(2 * p + 1, pcp, eB)
                    prev = (cp, ets)
                pcp, (eA, eB) = prev
                pv_cp(2 * p, pcp, eA)
                pv_cp(2 * p + 1, pcp, eB)

            # pair 0 leads: its first two chunk-pairs' scores/exp issue before
            # v_pass(0,0) so ACT has work the moment vt lands; its PV drains
            # those quarters as soon as each vt half-pass completes.
            e00 = sc_exp_cp(0, 0)
            e01 = sc_exp_cp(0, 1)
            v_pass(0, 0)
            for cp, (eA, eB) in ((0, e00), (1, e01)):
                pv_cp(0, cp, eA)
                pv_cp(1, cp, eB)
            e02 = sc_exp_cp(0, 2)
            e03 = sc_exp_cp(0, 3)
            v_pass(1, 0)
            for cp, (eA, eB) in ((2, e02), (3, e03)):
                pv_cp(0, cp, eA)
                pv_cp(1, cp, eB)
            pair(1, range(4))
            v_pass(0, 1)
            pair(2, range(4))
            pair(3, range(4))
            v_pass(1, 1)
            for p in range(4, NP):
                pair(p, range(4))

            # ---------------- output projection + residual + LN ----------
            wo8 = [load_w(wo, i, "wo") for i in range(NC)]
            if apply_gb:
                for i, t in enumerate((gamma, beta)):
                    nc.scalar.dma_start(
                        gb[:, i, :], bass.AP(tensor=t, offset=0, ap=[[0, P], [1, D]])
                    )
            for scc in range(4):
                rc = rp.tile([P, D], f32, tag="r", name="rc")
                nc.scalar.dma_start(rc, resid[scc * P:(scc + 1) * P, :])
                ps_o = big()
                for dc in range(NC):
                    for eh in range(2):
                        nc.tensor.matmul(
                            ps_o[:, eh, :],
                            xT[:, dc, scc * P:(scc + 1) * P],
                            wo8[dc][:, eh * SH:(eh + 1) * SH],
                            start=(dc == 0), stop=(dc == NC - 1),
                        )
                xl = rc  # LN runs in-place on the residual tile
                nc.vector.tensor_add(
                    out=xl, in0=ps_o.rearrange("p a b -> p (a b)"), in1=rc
                )
                stats = small.tile([P, 2, nc.vector.BN_STATS_DIM], f32, tag="stats",
                                   name="stats")
                for i in range(2):
                    nc.vector.bn_stats(stats[:, i, :], xl[:, i * SH:(i + 1) * SH])
                mv = small.tile([P, nc.vector.BN_AGGR_DIM], f32, tag="mv", name="mv")
                nc.vector.bn_aggr(mv, stats)
                std = small.tile([P, 1], f32, tag="std", name="std")
                nc.scalar.activation(
                    out=std, in_=mv[:, 1:2],
                    func=mybir.ActivationFunctionType.Sqrt,
                    bias=eps_t, scale=1.0,
                )
                rstd = small.tile([P, 1], f32, tag="rstd", name="rstd")
                nc.vector.reciprocal_approx_fast(rstd, std)
                nc.vector.tensor_scalar(
                    out=xl, in0=xl, scalar1=mv[:, 0:1], scalar2=rstd,
                    op0=mybir.AluOpType.subtract, op1=mybir.AluOpType.mult,
                )
                if apply_gb:
                    nc.vector.tensor_mul(out=xl, in0=xl, in1=gb[:, 0, :])
                    nc.vector.tensor_add(out=xl, in0=xl, in1=gb[:, 1, :])
                nc.sync.dma_start(out[scc * P:(scc + 1) * P, :], xl)

    nc.compile()
    return nc


def kernel(query, key, value, Wq, Wk, Wv, Wo, ln_gamma, ln_beta):
    global LAST_EXEC_NS
    apply_gb = not (
        np.all(np.asarray(ln_gamma) == 1.0) and np.all(np.asarray(ln_beta) == 0.0)
    )
    ck = ("nc", apply_gb)
    if ck not in _CACHE:
        _CACHE[ck] = _build(apply_gb)
    nc = _CACHE[ck]

    query = np.asarray(query, np.float32)
    key = np.asarray(key, np.float32)
    value = np.asarray(value, np.float32)
    wqT = np.ascontiguousarray(np.asarray(Wq, np.float32).T)
    wkT = np.ascontiguousarray(np.asarray(Wk, np.float32).T)
    wvT = np.ascontiguousarray(np.asarray(Wv, np.float32).T)
    woT = np.ascontiguousarray(np.asarray(Wo, np.float32).T)
    gamma = np.ascontiguousarray(np.asarray(ln_gamma, np.float32))
    beta = np.ascontiguousarray(np.asarray(ln_beta, np.float32))

    in_maps = []
    for core in range(NCORES):
        b, half = core // 2, core % 2
        sl = slice(half * SH, (half + 1) * SH)
        in_maps.append({
            "xqT": np.ascontiguousarray(query[b].T[:, sl]),
            "xkT": np.ascontiguousarray(key[b].T),
            "xvT": np.ascontiguousarray(value[b].T),
            "wq": wqT, "wk": wkT, "wv": wvT, "wo": woT,
            "resid": np.ascontiguousarray(query[b, sl]),
            "gamma": gamma, "beta": beta,
        })

    res = bass_utils.run_bass_kernel_spmd(
        nc, in_maps, core_ids=list(range(NCORES)), trace=TRACE
    )
    LAST_EXEC_NS = res.exec_time_ns

    out = np.empty((B, S, D), np.float32)
    for core in range(NCORES):
        b, half = core // 2, core % 2
        out[b, half * SH:(half + 1) * SH] = np.asarray(res.results[core]["out"])
    return out



# revision 3
# speedup vs baseline: 8.8564x; 1.0238x over previous
"""nn_MultiHeadAttention: fused MHA + residual + LayerNorm on 8 TRN2 NeuronCores.

Sharding: core = (batch b, query-half). Each core computes, for its batch:
  - Q projection for its 512 query rows, K/V projections for all 1024 keys
    (K/V work duplicated within a batch pair -> zero cross-core communication),
  - all 16 heads' attention for its query rows,
  - output projection + residual + LayerNorm for its rows.
Host concatenates the 8 [512, 1024] results into [4, 1024, 1024].

Schedule (single TileContext dataflow program, issue order = priority):
  K-proj (4 passes) -> Q-proj (2 passes) -> per head-pair: scores (row-packed
  2 heads across the 64-row PE groups) -> exp (ACT) -> PV, with the four
  V-projection passes and the O-projection interleaved into the attention
  stream so the ~70us of ACT exp work hides behind PE matmuls.

DMA: one dma_start per [128, 512..1024] chunk (2-4KB per partition line),
split across both HWDGE queues: SP carries weights + output, ACT carries
activations.  All matmuls f32r (1 cycle/row at N=512).
"""
import numpy as np

import concourse.bass as bass
import concourse.mybir as mybir
import concourse.tile as tile
from concourse import bacc, bass_utils

B, S, D, H, DK = 4, 1024, 1024, 16, 64
P = 128
SH = S // 2           # query rows per core
NC = D // P           # 8 chunks of 128 along any d-dimension
NP = H // 2           # 8 head pairs (one 128-dim chunk each)
NCORES = 8
EPS = 1e-6
f32 = mybir.dt.float32
f32r = mybir.dt.float32r

TRACE = False          # set by test.py to profile
LAST_EXEC_NS = None

_CACHE = {}


def _build(apply_gb):
    nc = bacc.Bacc("TRN2")
    xqT = nc.dram_tensor("xqT", [D, SH], f32, kind="ExternalInput")
    xkT = nc.dram_tensor("xkT", [D, S], f32, kind="ExternalInput")
    xvT = nc.dram_tensor("xvT", [D, S], f32, kind="ExternalInput")
    wq = nc.dram_tensor("wq", [D, D], f32, kind="ExternalInput")   # Wq.T  [in, out]
    wk = nc.dram_tensor("wk", [D, D], f32, kind="ExternalInput")
    wv = nc.dram_tensor("wv", [D, D], f32, kind="ExternalInput")
    wo = nc.dram_tensor("wo", [D, D], f32, kind="ExternalInput")   # Wo.T  [d, e]
    resid = nc.dram_tensor("resid", [SH, D], f32, kind="ExternalInput")
    gamma = nc.dram_tensor("gamma", [D], f32, kind="ExternalInput")
    beta = nc.dram_tensor("beta", [D], f32, kind="ExternalInput")
    out = nc.dram_tensor("out", [SH, D], f32, kind="ExternalOutput")

    with tile.TileContext(nc) as tc:
        with (
            tc.tile_pool(name="wpool", bufs=8) as wpool,
            tc.tile_pool(name="xs", bufs=10) as xsp,
            tc.tile_pool(name="xvp", bufs=8) as xvp,
            tc.tile_pool(name="persist", bufs=1) as persist,
            tc.tile_pool(name="expp", bufs=4) as expp,
            tc.tile_pool(name="rp", bufs=4) as rp,
            tc.tile_pool(name="small", bufs=2) as small,
            tc.tile_pool(name="psum", bufs=4, space="PSUM") as psum,
        ):
            # ---------------- persistent tiles ----------------
            kT = persist.tile([P, NC, S], f32r)       # [dim-in-pair, pair, sk]
            qT = persist.tile([P, NC, SH], f32r)      # [dim-in-pair, pair, sq]
            vt = persist.tile([P, NC, H, DK + 1], f32r)  # [sk-in-chunk, sk-chunk, (h, d|1)]
            xT = persist.tile([P, NC, SH], f32r)      # normalized attn out
            if apply_gb:
                gb = persist.tile([P, 2, D], f32)     # gamma/beta broadcast
            eps_t = persist.tile([P, 1], f32)

            nc.vector.memset(eps_t, EPS)
            nc.vector.memset(vt[:, :, :, DK:DK + 1].bitcast(f32), 1.0)  # ones col

            def load_w(w, i, nm, split_first=False):
                """One [128, 1024] weight chunk as a single 512KB DMA (SP q).
                split_first: head 64KB lands first so the first matmul can
                start before the rest of the chunk streams in."""
                wt = wpool.tile([P, D], f32r, tag="w", name=f"{nm}{i}")
                if split_first:
                    nc.sync.dma_start(
                        wt[:, 0:2 * P], w[i * P:(i + 1) * P, 0:2 * P].bitcast(f32r)
                    )
                    nc.sync.dma_start(
                        wt[:, 2 * P:], w[i * P:(i + 1) * P, 2 * P:].bitcast(f32r)
                    )
                else:
                    nc.sync.dma_start(wt, w[i * P:(i + 1) * P, :].bitcast(f32r))
                return wt

            def load_xh(x, i, col0, nm):
                """One [128, 512] half-chunk as a single 256KB DMA (ACT q)."""
                xc = xsp.tile([P, SH], f32r, tag="xs", name=nm)
                nc.scalar.dma_start(
                    xc, x[i * P:(i + 1) * P, col0:col0 + SH].bitcast(f32r)
                )
                return xc

            def big():
                return psum.tile([P, 2, SH], f32, tag="mm", name="big")

            # ---------------- K projection ----------------
            # kT[j, sk] = sum_i Wk.T[i, j] * xkT[i, sk]
            # 4 passes (sk-half x j-group); xk half-chunks stay live across
            # their half's two j-group passes.
            wk8 = []
            for sh in range(2):
                xk8 = []
                for jg in range(2):
                    ps_k = [big() for _ in range(2)]
                    for i in range(NC):
                        if sh == 0 and jg == 0:
                            wk8.append(load_w(wk, i, "wk", split_first=(i == 0)))
                        if jg == 0:
                            xk8.append(load_xh(xkT, i, sh * SH, "xk"))
                        for jj in range(4):
                            j = jg * 4 + jj
                            nc.tensor.matmul(
                                ps_k[jj // 2][:, jj % 2, :],
                                wk8[i][:, j * P:(j + 1) * P], xk8[i],
                                start=(i == 0), stop=(i == NC - 1),
                            )
                    for jj in range(4):
                        j = jg * 4 + jj
                        eng = nc.scalar.copy if jj % 2 == 0 else nc.vector.tensor_copy
                        eng(kT[:, j, sh * SH:(sh + 1) * SH], ps_k[jj // 2][:, jj % 2, :])

            # ---------------- Q projection ----------------
            wq8 = []
            xq8 = []
            for jg in range(2):
                ps_q = [big() for _ in range(2)]
                for i in range(NC):
                    if jg == 0:
                        wq8.append(load_w(wq, i, "wq"))
                        xq8.append(load_xh(xqT, i, 0, "xq"))
                    for jj in range(4):
                        j = jg * 4 + jj
                        nc.tensor.matmul(
                            ps_q[jj // 2][:, jj % 2, :],
                            wq8[i][:, j * P:(j + 1) * P], xq8[i],
                            start=(i == 0), stop=(i == NC - 1),
                        )
                for jj in range(4):
                    j = jg * 4 + jj
                    eng = nc.scalar.copy if jj % 2 == 0 else nc.vector.tensor_copy
                    eng(qT[:, j, :], ps_q[jj // 2][:, jj % 2, :])

            # ---------------- V projection (4 passes) + attention ----------
            # V pass (scg, dh): vt[sk in half scg, heads dh*8..dh*8+7].
            # xv halves are re-loaded per dh pass (cheaper than keeping 4MB
            # live); wv chunks stay resident across all four passes.
            wv8 = []

            def v_pass(scg, dh):
                ps_v = [big() for _ in range(2)]
                xvh = []
                for i in range(NC):
                    if scg == 0 and dh == 0:
                        wv8.append(load_w(wv, i, "wv"))
                    xc = xvp.tile([P, SH], f32r, tag="xv", name="xv")
                    nc.scalar.dma_start(
                        xc, xvT[i * P:(i + 1) * P, scg * SH:scg * SH + SH].bitcast(f32r)
                    )
                    xvh.append(xc)
                    for sl in range(4):
                        nc.tensor.matmul(
                            ps_v[sl // 2][:, sl % 2, :],
                            xvh[i][:, sl * P:(sl + 1) * P],
                            wv8[i][:, dh * SH:(dh + 1) * SH],
                            start=(i == 0), stop=(i == NC - 1),
                        )
                for sl in range(4):
                    sc = scg * 4 + sl
                    nc.vector.tensor_copy(
                        vt[:, sc, dh * 8:(dh + 1) * 8, :DK],
                        ps_v[sl // 2][:, sl % 2, :].rearrange("p (h d) -> p h d", d=DK),
                    )

            pv_state = {}

            def sc_exp_cp(p, cp):
                """Row-packed scores for heads (2p, 2p+1), sk chunks
                (2cp, 2cp+1), then exp. Returns (etA, etB)."""
                ps2 = [big() for _ in range(2)]
                for k in range(2):
                    c = 2 * cp + k
                    for a in range(2):
                        nc.tensor.matmul(
                            ps2[a][:, k, :],
                            kT[a * DK:(a + 1) * DK, p, c * P:(c + 1) * P],
                            qT[a * DK:(a + 1) * DK, p, :],
                            start=True, stop=True,
                        )
                ets = []
                for a in range(2):
                    et = expp.tile([P, 2, SH], f32r, tag="e", name="et")
                    nc.scalar.activation(
                        out=et, in_=ps2[a],
                        func=mybir.ActivationFunctionType.Exp,
                        scale=1.0 / np.sqrt(np.float32(DK)),
                    )
                    ets.append(et)
                return ets

            def pv_cp(h, cp, et):
                """Accumulate PV for head h over sk chunks (2cp, 2cp+1) in
                bank h%2 of its pair's accumulator tile; normalize after the
                last chunk."""
                a, p = h % 2, h // 2
                if p not in pv_state:
                    pv_state[p] = big()
                pvt = pv_state[p]
                for k in range(2):
                    c = 2 * cp + k
                    nc.tensor.matmul(
                        pvt[:DK + 1, a, :], vt[:, c, h, :], et[:, k, :],
                        start=(c == 0), stop=(c == NC - 1),
                    )
                if cp != 3:
                    return
                if a == 1:
                    del pv_state[p]
                sums_raw = small.tile([1, SH], f32, tag="sums_raw", name="sums_raw",
                                      bufs=1)
                nc.vector.tensor_copy(sums_raw, pvt[DK:DK + 1, a, :])
                sums = small.tile([1, SH], f32, tag="sums", name="sums")
                # approx reciprocal needs an SBUF input (bit-trick path)
                nc.vector.reciprocal_approx_fast(sums, sums_raw)
                rbc = small.tile([DK, SH], f32, tag="rbc", name="rbc")
                nc.gpsimd.partition_broadcast(rbc, sums)
                nc.vector.tensor_mul(
                    out=xT[a * DK:(a + 1) * DK, p, :], in0=pvt[:DK, a, :], in1=rbc
                )

            def pair(p, cps):
                # software-pipelined: scores/exp for chunk-pair cp+1 are
                # emitted before PV of cp, so the PE's static order never
                # makes the next exp wait a full pv+scores round-trip
                prev = None
                for cp in cps:
                    ets = sc_exp_cp(p, cp)
                    if prev is not None:
                        pcp, (eA, eB) = prev
                        pv_cp(2 * p, pcp, eA)
                        pv_cp(2 * p + 1, pcp, eB)
                    prev = (cp, ets)
                pcp, (eA, eB) = prev
                pv_cp(2 * p, pcp, eA)
                pv_cp(2 * p + 1, pcp, eB)

            # pair 0 leads: its first two chunk-pairs' scores/exp issue before
            # v_pass(0,0) so ACT has work the moment vt lands; its PV drains
            # those quarters as soon as each vt half-pass completes.
            e00 = sc_exp_cp(0, 0)
            e01 = sc_exp_cp(0, 1)
            v_pass(0, 0)
            for cp, (eA, eB) in ((0, e00), (1, e01)):
                pv_cp(0, cp, eA)
                pv_cp(1, cp, eB)
            e02 = sc_exp_cp(0, 2)
            e03 = sc_exp_cp(0, 3)
            v_pass(1, 0)
            for cp, (eA, eB) in ((2, e02), (3, e03)):
                pv_cp(0, cp, eA)
                pv_cp(1, cp, eB)
            pair(1, range(4))
            v_pass(0, 1)
            pair(2, range(4))
            pair(3, range(4))
            v_pass(1, 1)
            for p in range(4, NP):
                pair(p, range(4))

            # ---------------- output projection + residual + LN ----------
            wo8 = [load_w(wo, i, "wo") for i in range(NC)]
            if apply_gb:
                for i, t in enumerate((gamma, beta)):
                    nc.scalar.dma_start(
                        gb[:, i, :], bass.AP(tensor=t, offset=0, ap=[[0, P], [1, D]])
                    )
            rcs = []
            for scc in range(4):
                rc = rp.tile([P, D], f32, tag="r", name="rc")
                nc.scalar.dma_start(rc, resid[scc * P:(scc + 1) * P, :])
                rcs.append(rc)
            for scc in range(4):
                rc = rcs[scc]
                ps_o = big()
                for dc in range(NC):
                    for eh in range(2):
                        nc.tensor.matmul(
                            ps_o[:, eh, :],
                            xT[:, dc, scc * P:(scc + 1) * P],
                            wo8[dc][:, eh * SH:(eh + 1) * SH],
                            start=(dc == 0), stop=(dc == NC - 1),
                        )
                xl = rc  # LN runs in-place on the residual tile
                nc.vector.tensor_add(
                    out=xl, in0=ps_o.rearrange("p a b -> p (a b)"), in1=rc
                )
                stats = small.tile([P, 2, nc.vector.BN_STATS_DIM], f32, tag="stats",
                                   name="stats")
                for i in range(2):
                    nc.vector.bn_stats(stats[:, i, :], xl[:, i * SH:(i + 1) * SH])
                mv = small.tile([P, nc.vector.BN_AGGR_DIM], f32, tag="mv", name="mv")
                nc.vector.bn_aggr(mv, stats)
                std = small.tile([P, 1], f32, tag="std", name="std")
                nc.scalar.activation(
                    out=std, in_=mv[:, 1:2],
                    func=mybir.ActivationFunctionType.Sqrt,
                    bias=eps_t, scale=1.0,
                )
                rstd = small.tile([P, 1], f32, tag="rstd", name="rstd")
                nc.vector.reciprocal_approx_fast(rstd, std)
                nc.vector.tensor_scalar(
                    out=xl, in0=xl, scalar1=mv[:, 0:1], scalar2=rstd,
                    op0=mybir.AluOpType.subtract, op1=mybir.AluOpType.mult,
                )
                if apply_gb:
                    nc.vector.tensor_mul(out=xl, in0=xl, in1=gb[:, 0, :])
                    nc.vector.tensor_add(out=xl, in0=xl, in1=gb[:, 1, :])
                nc.sync.dma_start(out[scc * P:(scc + 1) * P, :], xl)

    nc.compile()
    return nc


def kernel(query, key, value, Wq, Wk, Wv, Wo, ln_gamma, ln_beta):
    global LAST_EXEC_NS
    apply_gb = not (
        np.all(np.asarray(ln_gamma) == 1.0) and np.all(np.asarray(ln_beta) == 0.0)
    )
    ck = ("nc", apply_gb)
    if ck not in _CACHE:
        _CACHE[ck] = _build(apply_gb)
    nc = _CACHE[ck]

    query = np.asarray(query, np.float32)
    key = np.asarray(key, np.float32)
    value = np.asarray(value, np.float32)
    wqT = np.ascontiguousarray(np.asarray(Wq, np.float32).T)
    wkT = np.ascontiguousarray(np.asarray(Wk, np.float32).T)
    wvT = np.ascontiguousarray(np.asarray(Wv, np.float32).T)
    woT = np.ascontiguousarray(np.asarray(Wo, np.float32).T)
    gamma = np.ascontiguousarray(np.asarray(ln_gamma, np.float32))
    beta = np.ascontiguousarray(np.asarray(ln_beta, np.float32))

    in_maps = []
    for core in range(NCORES):
        b, half = core // 2, core % 2
        sl = slice(half * SH, (half + 1) * SH)
        in_maps.append({
            "xqT": np.ascontiguousarray(query[b].T[:, sl]),
            "xkT": np.ascontiguousarray(key[b].T),
            "xvT": np.ascontiguousarray(value[b].T),
            "wq": wqT, "wk": wkT, "wv": wvT, "wo": woT,
            "resid": np.ascontiguousarray(query[b, sl]),
            "gamma": gamma, "beta": beta,
        })

    res = bass_utils.run_bass_kernel_spmd(
        nc, in_maps, core_ids=list(range(NCORES)), trace=TRACE
    )
    LAST_EXEC_NS = res.exec_time_ns

    out = np.empty((B, S, D), np.float32)
    for core in range(NCORES):
        b, half = core // 2, core % 2
        out[b, half * SH:(half + 1) * SH] = np.asarray(res.results[core]["out"])
    return out

